# revision 16
# baseline (speedup 1.0000x reference)
"""ClassCapsule dynamic-routing kernel for 8 Trainium2 NeuronCores.

Problem (hardcoded shapes):
    x:    [64, 2048, 16]  fp32
    W:    [2048, 16, 1024] fp32
    bias: [64, 16]        fp32
    out:  [64, 64, 16]    fp32  (squeezed v after 3 routing iterations)

Strategy (in_caps-sharded, W resident in SBUF, u_hat recomputed per
iteration, per-iteration AllReduce of the small s tensor):
  - in_caps=2048 split across 8 cores (256 each); every core holds the
    full batch B=64.  W slice (bf16) lives in SBUF for the whole kernel,
    so u_hat is recomputed on the PE each routing iteration instead of
    being bounced through DRAM.  Total HBM traffic is ~15 MB/core.
  - u_hat tiles [128=(i8,b16), 1024=(d16,n64)] come from block-diagonal
    matmuls: lhsT = block-diag x (8 in_caps share K=128=(i8,e16)),
    rhs = W block.  Column order (d major, n minor) keeps the free-dim
    broadcast of c packed so DVE runs in 2x bf16 mode.
  - iteration 0 (uniform c): s0 = sum_i u/64 collapses into a dense
    x^T @ W matmul - no u_hat materialization at all.
  - routing: agreement = u*v reduced over d via a halving add tree
    (DVE, bf16), softmax over n (ACT exp + DVE), weighted sum over i
    via selector matmuls on the PE accumulating in PSUM.
  - s [64,1024] partials are AllReduced (collective_compute) across the
    8 cores each iteration; every core computes squash/v redundantly.
"""

import numpy as np
import ml_dtypes

import concourse.bass as bass
import concourse.tile as tile
from concourse import bacc, mybir
from concourse.bass_utils import run_bass_kernel_spmd

# ---------------------------------------------------------------- constants
B, IC, E = 64, 2048, 16          # batch, in_caps, in_dim
NCAP, D = 64, 16                 # n_caps, cap_dim
ND = NCAP * D                    # 1024
CORES = 8
ICL = IC // CORES                # 256 local in_caps
NB = ICL // 8                    # 32 blocks of 8 in_caps
BC = 4                           # batch chunks of 16
EPS = 1e-7

FP = mybir.dt.float32
BF = mybir.dt.bfloat16
BF_NP = ml_dtypes.bfloat16


def _host_prep(x, W, bias):
    """Per-core host-side tensors (bf16, (d,n) column order)."""
    # W columns reordered from (n,d) to (d,n): new_col = d*64 + n
    W_dn = W.reshape(IC, E, NCAP, D).transpose(0, 1, 3, 2).reshape(IC, E, ND)

    w_all, xbd_all, xd_all = [], [], []
    for c in range(CORES):
        sl = slice(c * ICL, (c + 1) * ICL)
        W_c = W_dn[sl]                                   # [256, 16, 1024]
        # -> [128=(i8,e16) partitions, 32 blocks, 1024]
        w_all.append(np.ascontiguousarray(
            W_c.reshape(NB, 8, E, ND).transpose(1, 2, 0, 3).reshape(128, NB, ND)
        ).astype(BF_NP))

        x_c = x[:, sl]                                   # [64, 256, 16]
        # block-diagonal lhsT: [128=(i8,e16), blk, bc, 128=(i8,b16)]
        x_r = x_c.reshape(BC, 16, NB, 8, E).transpose(3, 4, 2, 0, 1)
        arr = np.zeros((8, E, NB, BC, 8, 16), dtype=np.float32)
        for s in range(8):
            arr[s, :, :, :, s, :] = x_r[s]
        xbd_all.append(arr.reshape(128, NB, BC, 128).astype(BF_NP))

        # dense lhsT for iter-0 direct sum: [128=(i8,e16), blk, 128(m: b64 pad)]
        xd = np.zeros((128, NB, 128), dtype=np.float32)
        xd[:, :, :B] = x_c.reshape(B, NB, 8, E).transpose(2, 3, 1, 0).reshape(128, NB, B)
        xd_all.append(xd.astype(BF_NP))

    # selector weights, one per batch chunk: sel[bc][k=(i8,b16), m=128] with
    # m = bc*16 + (k%16) set to 1  (M=128 keeps free-weight-load enabled)
    sels = np.zeros((BC, 128, 128), dtype=np.float32)
    for bc in range(BC):
        k = np.arange(128)
        sels[bc, k, bc * 16 + (k % 16)] = 1.0
    sels = np.ascontiguousarray(sels.transpose(1, 0, 2)).astype(BF_NP)  # [128, BC, 128]

    # bias in (d,n) order, tiled over batch: [64, 1024]
    bias_dn = np.ascontiguousarray(bias.T).reshape(1, ND)       # [d,n] flat
    bias_f = np.tile(bias_dn, (B, 1)).astype(np.float32)
    return w_all, xbd_all, xd_all, sels, bias_f


def _build_program():
    nc = bacc.Bacc("TRN2", target_bir_lowering=False, num_devices=CORES)

    w_d = nc.dram_tensor("w_d", [128, NB, ND], BF, kind="ExternalInput")
    xbd_d = nc.dram_tensor("xbd_d", [128, NB, BC, 128], BF, kind="ExternalInput")
    xd_d = nc.dram_tensor("xd_d", [128, NB, 128], BF, kind="ExternalInput")
    sel_d = nc.dram_tensor("sel_d", [128, BC, 128], BF, kind="ExternalInput")
    bias_d = nc.dram_tensor("bias_d", [B, ND], FP, kind="ExternalInput")
    v_out = nc.dram_tensor("v_out", [B, ND], FP, kind="ExternalOutput")

    v_scr = nc.dram_tensor("v_scr", [B, ND], BF)     # bounce for vb build

    with tile.TileContext(nc) as tc:
        with (
            tc.tile_pool(name="consts", bufs=1) as cp,
            tc.tile_pool(name="ubf", bufs=3) as up,       # [128, 4096] grouped
            tc.tile_pool(name="tmp", bufs=3) as tp,       # [128, 4096] tmp+ws ring
            tc.tile_pool(name="vb", bufs=1) as vbp,
            tc.tile_pool(name="smalls", bufs=2) as sp,
            tc.tile_pool(name="sq", bufs=1) as qp,
            tc.tile_pool(name="ups", bufs=3, space="PSUM") as psp,
            tc.tile_pool(name="sps", bufs=1, space="PSUM") as psa,
            tc.tile_pool(name="bstate", bufs=1) as bsp,
            tc.tile_pool(name="dram", bufs=2, space="DRAM") as dp,
        ):
            # ---- resident tensors
            w_sb = cp.tile([128, NB * ND], BF)
            nc.sync.dma_start(out=w_sb, in_=w_d[:, :, :])
            xbd_sb = cp.tile([128, NB * BC * 128], BF)
            nc.sync.dma_start(out=xbd_sb, in_=xbd_d[:, :, :, :])
            # iter-0 dense lhsT borrows a u_g ring slot (same shape); the
            # ring recycles it once iter 0 is done
            xd_sb = up.tile([128, NB * 128], BF, tag="u_g")
            nc.sync.dma_start(out=xd_sb, in_=xd_d[:, :, :])
            sel_sb = cp.tile([128, BC * 128], BF)
            nc.sync.dma_start(out=sel_sb, in_=sel_d[:, :, :])
            bias_sb = cp.tile([B, ND], FP)
            nc.sync.dma_start(out=bias_sb, in_=bias_d[:, :])
            eps_t = cp.tile([B, 1], FP)
            nc.vector.memset(eps_t, EPS)

            # routing logits state: [128=(i8,b16), bc*blk*n]
            b_all = bsp.tile([128, BC * NB * NCAP], FP)

            def w_blk(blk, h):
                return w_sb[:, blk * ND + h * 512:blk * ND + (h + 1) * 512]

            # ---------------- AllReduce s -> (scale,bias) -> squash -> v
            def reduce_squash_v(s_ps, scale, last):
                s_par = qp.tile([B, ND], FP, tag="q1")
                nc.vector.tensor_copy(out=s_par, in_=s_ps[0:B, :])
                s_in = dp.tile([B, ND], FP, tag="cc_in")
                nc.sync.dma_start(out=s_in[:], in_=s_par)
                s_red = dp.tile([B, ND], FP, tag="cc_out")
                nc.gpsimd.collective_compute(
                    "AllReduce",
                    mybir.AluOpType.add,
                    replica_groups=[list(range(CORES))],
                    ins=[s_in[:].opt()],
                    outs=[s_red[:].opt()],
                )
                s_glob = qp.tile([B, ND], FP, tag="q2")
                nc.sync.dma_start(out=s_glob, in_=s_red[:])
                # s = s_glob*scale + bias
                s_sb = qp.tile([B, ND], FP, tag="q1")
                nc.vector.scalar_tensor_tensor(
                    out=s_sb, in0=s_glob, scalar=float(scale), in1=bias_sb,
                    op0=mybir.AluOpType.mult, op1=mybir.AluOpType.add)
                sqr = qp.tile([B, ND], FP, tag="q2")
                nc.scalar.square(out=sqr, in_=s_sb)
                nsq = sp.tile([B, NCAP], FP, tag="nsq")
                nc.vector.reduce_sum(
                    out=nsq, in_=sqr.rearrange("p (d n) -> p n d", d=D),
                    axis=mybir.AxisListType.X)
                norm = sp.tile([B, NCAP], FP, tag="norm")
                nc.scalar.activation(out=norm, in_=nsq,
                                     func=mybir.ActivationFunctionType.Sqrt,
                                     bias=eps_t[:, :], scale=1.0)
                den = sp.tile([B, NCAP], FP, tag="den")
                nc.vector.scalar_tensor_tensor(
                    out=den, in0=nsq, scalar=float(EPS + 1.0), in1=norm,
                    op0=mybir.AluOpType.add, op1=mybir.AluOpType.mult)
                rden = sp.tile([B, NCAP], FP, tag="rden")
                nc.vector.reciprocal(out=rden, in_=den)
                fac = sp.tile([B, NCAP], FP, tag="fac")
                nc.vector.scalar_tensor_tensor(
                    out=fac, in0=nsq, scalar=float(EPS), in1=rden,
                    op0=mybir.AluOpType.add, op1=mybir.AluOpType.mult)
                v_sb = qp.tile([B, ND], FP, tag="q2")
                fac_b = bass.AP(tensor=fac.tensor, offset=fac.offset,
                                ap=[list(fac.ap[0]), [0, D], list(fac.ap[1])])
                nc.vector.tensor_mul(
                    v_sb.rearrange("p (d n) -> p d n", d=D),
                    s_sb.rearrange("p (d n) -> p d n", d=D),
                    fac_b)
                if last:
                    nc.sync.dma_start(out=v_out[:, :], in_=v_sb)
                    return None
                v_bf = qp.tile([B, ND], BF, tag="q3")
                nc.vector.tensor_copy(out=v_bf, in_=v_sb)
                nc.sync.dma_start(out=v_scr[:, :], in_=v_bf)
                # vb_all[128=(i8,b16), (bc,d,n)]: v rows bc*16+b_lo, bcast i8
                vb = vbp.tile([128, BC * ND], BF, tag="vb")
                for bc in range(BC):
                    src = bass.AP(tensor=v_scr, offset=bc * 16 * ND,
                                  ap=[[0, 8], [ND, 16], [1, ND]])
                    nc.sync.dma_start(
                        out=vb[:, bc * ND:(bc + 1) * ND], in_=src)
                return vb

            # ================= iter 0: s0 = (1/64) sum_i u  ================
            s_ps = psa.tile([128, ND], FP, tag="s_acc")
            for blk in range(NB):
                for h in range(2):
                    nc.tensor.matmul(
                        s_ps[:, h * 512:(h + 1) * 512],
                        xd_sb[:, blk * 128:(blk + 1) * 128],
                        w_blk(blk, h),
                        start=(blk == 0), stop=(blk == NB - 1))
            vb = reduce_squash_v(s_ps, 1.0 / NCAP, last=False)

            # ================= routing iterations 1 and 2 =================
            for it in (1, 2):
                s_ps = psa.tile([128, ND], FP, tag="s_acc")
                for blk in range(NB):
                    u_g = up.tile([128, BC * ND], BF, tag="u_g")
                    for bc in range(BC):
                        u_ps = psp.tile([128, ND], FP, tag="u_ps")
                        lhs = xbd_sb[:, (blk * BC + bc) * 128:
                                     (blk * BC + bc + 1) * 128]
                        for h in range(2):
                            nc.tensor.matmul(
                                u_ps[:, h * 512:(h + 1) * 512],
                                lhs, w_blk(blk, h),
                                start=True, stop=True)
                        nc.scalar.copy(
                            out=u_g[:, bc * ND:(bc + 1) * ND], in_=u_ps)
                    # tmp = u * v  (bf16, packed -> 2x DVE)
                    tmp = tp.tile([128, BC * ND], BF, tag="tmp")
                    nc.vector.tensor_mul(tmp, u_g, vb)
                    # reduce over d: halving add tree on [p, bc, (d n)]
                    t3 = tmp.rearrange("p (c f) -> p c f", c=BC)
                    for half in (512, 256, 128, 64):
                        src_hi = bass.AP(
                            tensor=tmp.tensor, offset=tmp.offset + half,
                            ap=[list(tmp.ap[0]), [ND, BC], [1, half]])
                        if half > 64:
                            nc.vector.tensor_add(
                                t3[:, :, 0:half], t3[:, :, 0:half], src_hi)
                        else:
                            # final add -> b state (fp32)
                            b_dst = bass.AP(
                                tensor=b_all.tensor,
                                offset=b_all.offset + blk * NCAP,
                                ap=[list(b_all.ap[0]), [NB * NCAP, BC],
                                    [1, NCAP]])
                            if it == 1:
                                nc.vector.tensor_add(
                                    b_dst, t3[:, :, 0:64], src_hi)
                            else:
                                agr = sp.tile([128, BC * NCAP], FP, tag="agr")
                                a3 = agr.rearrange("p (c n) -> p c n", c=BC)
                                nc.vector.tensor_add(
                                    a3, t3[:, :, 0:64], src_hi)
                                nc.gpsimd.tensor_add(b_dst, b_dst, a3)
                    # softmax over n (free axis): per-chunk exp with fused
                    # accumulation (keeps the Z reduce off the DVE)
                    c_un = sp.tile([128, BC * NCAP], BF, tag="c_un")
                    zsum = sp.tile([128, BC], FP, tag="zsum")
                    for bc in range(BC):
                        nc.scalar.activation(
                            out=c_un[:, bc * NCAP:(bc + 1) * NCAP],
                            in_=b_all[:, (bc * NB + blk) * NCAP:
                                      (bc * NB + blk + 1) * NCAP],
                            func=mybir.ActivationFunctionType.Exp,
                            accum_out=zsum[:, bc:bc + 1])
                    rec = sp.tile([128, BC], BF, tag="rec")
                    with nc.allow_low_precision(reason="1/Z in bf16 is fine for softmax scale"):
                        nc.vector.reciprocal(out=rec, in_=zsum)
                    c_bf = sp.tile([128, BC * NCAP], BF, tag="c_bf")
                    rec_b = bass.AP(tensor=rec.tensor, offset=rec.offset,
                                    ap=[list(rec.ap[0]), [1, BC], [0, NCAP]])
                    nc.gpsimd.tensor_mul(
                        c_bf.rearrange("p (c n) -> p c n", c=BC),
                        c_un.rearrange("p (c n) -> p c n", c=BC),
                        rec_b)
                    # w = u * c (c bcast over d; last dim packed -> 2x DVE)
                    w_g = tp.tile([128, BC * ND], BF, tag="tmp")
                    c_b = bass.AP(tensor=c_bf.tensor, offset=c_bf.offset,
                                  ap=[list(c_bf.ap[0]), [NCAP, BC], [0, D],
                                      [1, NCAP]])
                    nc.vector.tensor_mul(
                        w_g.rearrange("p (c d n) -> p c d n", c=BC, d=D),
                        u_g.rearrange("p (c d n) -> p c d n", c=BC, d=D),
                        c_b)
                    # s += sel_bc^T w   (accumulate over blocks in PSUM)
                    for bc in range(BC):
                        for h in range(2):
                            nc.tensor.matmul(
                                s_ps[:, h * 512:(h + 1) * 512],
                                sel_sb[:, bc * 128:(bc + 1) * 128],
                                w_g[:, bc * ND + h * 512:bc * ND + (h + 1) * 512],
                                start=(blk == 0 and bc == 0),
                                stop=(blk == NB - 1 and bc == BC - 1),
                                skip_group_check=True)
                vb = reduce_squash_v(s_ps, 1.0, last=(it == 2))

    nc.compile()
    return nc


_CACHED = {}


def _get_program():
    if "nc" not in _CACHED:
        _CACHED["nc"] = _build_program()
    return _CACHED["nc"]


def kernel(x, W, bias):
    x = np.asarray(x, dtype=np.float32)
    W = np.asarray(W, dtype=np.float32)
    bias = np.asarray(bias, dtype=np.float32)

    w_all, xbd_all, xd_all, sels, bias_f = _host_prep(x, W, bias)
    nc = _get_program()

    in_maps = []
    for c in range(CORES):
        in_maps.append({
            "w_d": w_all[c],
            "xbd_d": xbd_all[c],
            "xd_d": xd_all[c],
            "sel_d": sels,
            "bias_d": bias_f,
        })
    res = run_bass_kernel_spmd(nc, in_maps, core_ids=list(range(CORES)))
    _CACHED["last_results"] = res
    # v_out is replicated; columns are (d,n) ordered -> [b, n, d]
    v = res.results[0]["v_out"].reshape(B, D, NCAP).transpose(0, 2, 1)
    return np.ascontiguousarray(v)


# revision 18
# speedup vs baseline: 1.2394x; 1.2394x over previous
"""ClassCapsule dynamic-routing kernel for 8 Trainium2 NeuronCores.

Problem (hardcoded shapes):
    x:    [64, 2048, 16]  fp32
    W:    [2048, 16, 1024] fp32
    bias: [64, 16]        fp32
    out:  [64, 64, 16]    fp32  (squeezed v after 3 routing iterations)

Strategy (in_caps-sharded, W resident in SBUF, u_hat recomputed per
iteration, per-iteration AllReduce of the small s tensor):
  - in_caps=2048 split across 8 cores (256 each); every core holds the
    full batch B=64.  W slice (bf16) lives in SBUF for the whole kernel,
    so u_hat is recomputed on the PE each routing iteration instead of
    being bounced through DRAM.  Total HBM traffic is ~15 MB/core.
  - u_hat tiles [128=(i8,b16), 1024=(d16,n64)] come from block-diagonal
    matmuls: lhsT = block-diag x (8 in_caps share K=128=(i8,e16)),
    rhs = W block.  Column order (d major, n minor) keeps the free-dim
    broadcast of c packed so DVE runs in 2x bf16 mode.
  - iteration 0 (uniform c): s0 = sum_i u/64 collapses into a dense
    x^T @ W matmul - no u_hat materialization at all.
  - routing: agreement = u*v reduced over d via a halving add tree
    (DVE, bf16), softmax over n (ACT exp + DVE), weighted sum over i
    via selector matmuls on the PE accumulating in PSUM.
  - s [64,1024] partials are AllReduced (collective_compute) across the
    8 cores each iteration; every core computes squash/v redundantly.
"""

import numpy as np
import ml_dtypes

import concourse.bass as bass
import concourse.tile as tile
from concourse import bacc, mybir
from concourse.bass_utils import run_bass_kernel_spmd

# ---------------------------------------------------------------- constants
B, IC, E = 64, 2048, 16          # batch, in_caps, in_dim
NCAP, D = 64, 16                 # n_caps, cap_dim
ND = NCAP * D                    # 1024
CORES = 8
ICL = IC // CORES                # 256 local in_caps
NB = ICL // 8                    # 32 blocks of 8 in_caps
BC = 4                           # batch chunks of 16
EPS = 1e-7

FP = mybir.dt.float32
BF = mybir.dt.bfloat16
BF_NP = ml_dtypes.bfloat16


def _host_prep(x, W, bias):
    """Per-core host-side tensors (bf16, (d,n) column order)."""
    # W columns reordered from (n,d) to (d,n): new_col = d*64 + n
    W_dn = W.reshape(IC, E, NCAP, D).transpose(0, 1, 3, 2).reshape(IC, E, ND)

    w_all, xbd_all, xd_all = [], [], []
    for c in range(CORES):
        sl = slice(c * ICL, (c + 1) * ICL)
        W_c = W_dn[sl]                                   # [256, 16, 1024]
        # -> [128=(i8,e16) partitions, 32 blocks, 1024]
        w_all.append(np.ascontiguousarray(
            W_c.reshape(NB, 8, E, ND).transpose(1, 2, 0, 3).reshape(128, NB, ND)
        ).astype(BF_NP))

        x_c = x[:, sl]                                   # [64, 256, 16]
        # block-diagonal lhsT: [128=(i8,e16), blk, bc, 128=(i8,b16)]
        x_r = x_c.reshape(BC, 16, NB, 8, E).transpose(3, 4, 2, 0, 1)
        arr = np.zeros((8, E, NB, BC, 8, 16), dtype=np.float32)
        for s in range(8):
            arr[s, :, :, :, s, :] = x_r[s]
        xbd_all.append(arr.reshape(128, NB, BC, 128).astype(BF_NP))

        # dense lhsT for iter-0 direct sum: [128=(i8,e16), blk, 128(m: b64 pad)]
        xd = np.zeros((128, NB, 128), dtype=np.float32)
        xd[:, :, :B] = x_c.reshape(B, NB, 8, E).transpose(2, 3, 1, 0).reshape(128, NB, B)
        xd_all.append(xd.astype(BF_NP))

    # selector weights, one per batch chunk: sel[bc][k=(i8,b16), m=128] with
    # m = bc*16 + (k%16) set to 1  (M=128 keeps free-weight-load enabled)
    sels = np.zeros((BC, 128, 128), dtype=np.float32)
    for bc in range(BC):
        k = np.arange(128)
        sels[bc, k, bc * 16 + (k % 16)] = 1.0
    sels = np.ascontiguousarray(sels.transpose(1, 0, 2)).astype(BF_NP)  # [128, BC, 128]

    # bias in (d,n) order, tiled over batch: [64, 1024]
    bias_dn = np.ascontiguousarray(bias.T).reshape(1, ND)       # [d,n] flat
    bias_f = np.tile(bias_dn, (B, 1)).astype(np.float32)
    return w_all, xbd_all, xd_all, sels, bias_f


def _build_program():
    nc = bacc.Bacc("TRN2", target_bir_lowering=False, num_devices=CORES)

    w_d = nc.dram_tensor("w_d", [128, NB, ND], BF, kind="ExternalInput")
    xbd_d = nc.dram_tensor("xbd_d", [128, NB, BC, 128], BF, kind="ExternalInput")
    xd_d = nc.dram_tensor("xd_d", [128, NB, 128], BF, kind="ExternalInput")
    sel_d = nc.dram_tensor("sel_d", [128, BC, 128], BF, kind="ExternalInput")
    bias_d = nc.dram_tensor("bias_d", [B, ND], FP, kind="ExternalInput")
    v_out = nc.dram_tensor("v_out", [B, ND], FP, kind="ExternalOutput")

    v_scr = nc.dram_tensor("v_scr", [B, ND], BF)     # bounce for vb build

    with tile.TileContext(nc) as tc:
        with (
            tc.tile_pool(name="consts", bufs=1) as cp,
            tc.tile_pool(name="ubf", bufs=3) as up,       # [128, 4096] grouped
            tc.tile_pool(name="tmp", bufs=3) as tp,       # [128, 4096] tmp+ws ring
            tc.tile_pool(name="vb", bufs=1) as vbp,
            tc.tile_pool(name="smalls", bufs=2) as sp,
            tc.tile_pool(name="sq", bufs=1) as qp,
            tc.tile_pool(name="ups", bufs=3, space="PSUM") as psp,
            tc.tile_pool(name="sps", bufs=1, space="PSUM") as psa,
            tc.tile_pool(name="bstate", bufs=1) as bsp,
            tc.tile_pool(name="dram", bufs=2, space="DRAM") as dp,
        ):
            # ---- resident tensors
            w_sb = cp.tile([128, NB * ND], BF)
            nc.sync.dma_start(out=w_sb, in_=w_d[:, :, :])
            xbd_sb = cp.tile([128, NB * BC * 128], BF)
            nc.sync.dma_start(out=xbd_sb, in_=xbd_d[:, :, :, :])
            # iter-0 dense lhsT borrows a u_g ring slot (same shape); the
            # ring recycles it once iter 0 is done
            xd_sb = up.tile([128, NB * 128], BF, tag="u_g")
            nc.sync.dma_start(out=xd_sb, in_=xd_d[:, :, :])
            sel_sb = cp.tile([128, BC * 128], BF)
            nc.sync.dma_start(out=sel_sb, in_=sel_d[:, :, :])
            bias_sb = cp.tile([B, ND], FP)
            nc.sync.dma_start(out=bias_sb, in_=bias_d[:, :])
            eps_t = cp.tile([B, 1], FP)
            nc.vector.memset(eps_t, EPS)

            # routing logits state: [128=(i8,b16), bc*blk*n]
            b_all = bsp.tile([128, BC * NB * NCAP], FP)

            def w_blk(blk, h):
                return w_sb[:, blk * ND + h * 512:blk * ND + (h + 1) * 512]

            # ---------------- AllReduce s -> (scale,bias) -> squash -> v
            def reduce_squash_v(s_ps, scale, last):
                s_par = qp.tile([B, ND], FP, tag="q1")
                nc.vector.tensor_copy(out=s_par, in_=s_ps[0:B, :])
                s_in = dp.tile([B, ND], FP, tag="cc_in")
                nc.sync.dma_start(out=s_in[:], in_=s_par)
                s_red = dp.tile([B, ND], FP, tag="cc_out")
                nc.gpsimd.collective_compute(
                    "AllReduce",
                    mybir.AluOpType.add,
                    replica_groups=[list(range(CORES))],
                    ins=[s_in[:].opt()],
                    outs=[s_red[:].opt()],
                )
                s_glob = qp.tile([B, ND], FP, tag="q2")
                nc.sync.dma_start(out=s_glob, in_=s_red[:])
                # s = s_glob*scale + bias
                s_sb = qp.tile([B, ND], FP, tag="q1")
                nc.vector.scalar_tensor_tensor(
                    out=s_sb, in0=s_glob, scalar=float(scale), in1=bias_sb,
                    op0=mybir.AluOpType.mult, op1=mybir.AluOpType.add)
                sqr = qp.tile([B, ND], FP, tag="q2")
                nc.scalar.square(out=sqr, in_=s_sb)
                nsq = sp.tile([B, NCAP], FP, tag="nsq")
                nc.vector.reduce_sum(
                    out=nsq, in_=sqr.rearrange("p (d n) -> p n d", d=D),
                    axis=mybir.AxisListType.X)
                norm = sp.tile([B, NCAP], FP, tag="norm")
                nc.scalar.activation(out=norm, in_=nsq,
                                     func=mybir.ActivationFunctionType.Sqrt,
                                     bias=eps_t[:, :], scale=1.0)
                den = sp.tile([B, NCAP], FP, tag="den")
                nc.vector.scalar_tensor_tensor(
                    out=den, in0=nsq, scalar=float(EPS + 1.0), in1=norm,
                    op0=mybir.AluOpType.add, op1=mybir.AluOpType.mult)
                rden = sp.tile([B, NCAP], FP, tag="rden")
                nc.vector.reciprocal(out=rden, in_=den)
                fac = sp.tile([B, NCAP], FP, tag="fac")
                nc.vector.scalar_tensor_tensor(
                    out=fac, in0=nsq, scalar=float(EPS), in1=rden,
                    op0=mybir.AluOpType.add, op1=mybir.AluOpType.mult)
                v_sb = qp.tile([B, ND], FP, tag="q2")
                fac_b = bass.AP(tensor=fac.tensor, offset=fac.offset,
                                ap=[list(fac.ap[0]), [0, D], list(fac.ap[1])])
                nc.vector.tensor_mul(
                    v_sb.rearrange("p (d n) -> p d n", d=D),
                    s_sb.rearrange("p (d n) -> p d n", d=D),
                    fac_b)
                if last:
                    nc.sync.dma_start(out=v_out[:, :], in_=v_sb)
                    return None
                v_bf = qp.tile([B, ND], BF, tag="q3")
                nc.vector.tensor_copy(out=v_bf, in_=v_sb)
                nc.sync.dma_start(out=v_scr[:, :], in_=v_bf)
                # vb_all[128=(i8,b16), (bc,d,n)]: v rows bc*16+b_lo, bcast i8
                vb = vbp.tile([128, BC * ND], BF, tag="vb")
                for bc in range(BC):
                    src = bass.AP(tensor=v_scr, offset=bc * 16 * ND,
                                  ap=[[0, 8], [ND, 16], [1, ND]])
                    nc.sync.dma_start(
                        out=vb[:, bc * ND:(bc + 1) * ND], in_=src)
                return vb

            # ================= iter 0: s0 = (1/64) sum_i u  ================
            s_ps = psa.tile([128, ND], FP, tag="s_acc")
            for blk in range(NB):
                for h in range(2):
                    nc.tensor.matmul(
                        s_ps[:, h * 512:(h + 1) * 512],
                        xd_sb[:, blk * 128:(blk + 1) * 128],
                        w_blk(blk, h),
                        start=(blk == 0), stop=(blk == NB - 1))
            vb = reduce_squash_v(s_ps, 1.0 / NCAP, last=False)

            # ================= routing iterations 1 and 2 =================
            for it in (1, 2):
                s_ps = psa.tile([128, ND], FP, tag="s_acc")
                for blk in range(NB):
                    u_g = up.tile([128, BC * ND], BF, tag="u_g")
                    for bc in range(BC):
                        u_ps = psp.tile([128, ND], FP, tag="u_ps")
                        lhs = xbd_sb[:, (blk * BC + bc) * 128:
                                     (blk * BC + bc + 1) * 128]
                        for h in range(2):
                            nc.tensor.matmul(
                                u_ps[:, h * 512:(h + 1) * 512],
                                lhs, w_blk(blk, h),
                                start=True, stop=True)
                        nc.scalar.copy(
                            out=u_g[:, bc * ND:(bc + 1) * ND], in_=u_ps)
                    # tmp = u * v  (bf16, packed -> 2x DVE)
                    tmp = tp.tile([128, BC * ND], BF, tag="tmp")
                    nc.vector.tensor_mul(tmp, u_g, vb)
                    # reduce over d: halving add tree on [p, bc, (d n)]
                    t3 = tmp.rearrange("p (c f) -> p c f", c=BC)
                    for half in (512, 256, 128, 64):
                        src_hi = bass.AP(
                            tensor=tmp.tensor, offset=tmp.offset + half,
                            ap=[list(tmp.ap[0]), [ND, BC], [1, half]])
                        if half > 64:
                            nc.vector.tensor_add(
                                t3[:, :, 0:half], t3[:, :, 0:half], src_hi)
                        else:
                            # final add -> b state (fp32)
                            b_dst = bass.AP(
                                tensor=b_all.tensor,
                                offset=b_all.offset + blk * NCAP,
                                ap=[list(b_all.ap[0]), [NB * NCAP, BC],
                                    [1, NCAP]])
                            if it == 1:
                                nc.vector.tensor_add(
                                    b_dst, t3[:, :, 0:64], src_hi)
                            else:
                                agr = sp.tile([128, BC * NCAP], FP, tag="agr")
                                a3 = agr.rearrange("p (c n) -> p c n", c=BC)
                                nc.vector.tensor_add(
                                    a3, t3[:, :, 0:64], src_hi)
                                nc.vector.tensor_add(b_dst, b_dst, a3)
                    # softmax over n (free axis): exp, Z, recip, scale
                    b_src = bass.AP(
                        tensor=b_all.tensor,
                        offset=b_all.offset + blk * NCAP,
                        ap=[list(b_all.ap[0]), [NB * NCAP, BC], [1, NCAP]])
                    c_un = sp.tile([128, BC * NCAP], BF, tag="c_un")
                    nc.scalar.activation(
                        out=c_un.rearrange("p (c n) -> p c n", c=BC),
                        in_=b_src, func=mybir.ActivationFunctionType.Exp)
                    zsum = sp.tile([128, BC], FP, tag="zsum")
                    nc.vector.reduce_sum(
                        out=zsum, in_=c_un.rearrange("p (c n) -> p c n", c=BC),
                        axis=mybir.AxisListType.X)
                    rec = sp.tile([128, BC], BF, tag="rec")
                    with nc.allow_low_precision(reason="1/Z in bf16 is fine for softmax scale"):
                        nc.vector.reciprocal(out=rec, in_=zsum)
                    c_bf = sp.tile([128, BC * NCAP], BF, tag="c_bf")
                    rec_b = bass.AP(tensor=rec.tensor, offset=rec.offset,
                                    ap=[list(rec.ap[0]), [1, BC], [0, NCAP]])
                    nc.gpsimd.tensor_mul(
                        c_bf.rearrange("p (c n) -> p c n", c=BC),
                        c_un.rearrange("p (c n) -> p c n", c=BC),
                        rec_b)
                    # w = u * c (c bcast over d; last dim packed -> 2x DVE)
                    w_g = tp.tile([128, BC * ND], BF, tag="tmp")
                    c_b = bass.AP(tensor=c_bf.tensor, offset=c_bf.offset,
                                  ap=[list(c_bf.ap[0]), [NCAP, BC], [0, D],
                                      [1, NCAP]])
                    nc.vector.tensor_mul(
                        w_g.rearrange("p (c d n) -> p c d n", c=BC, d=D),
                        u_g.rearrange("p (c d n) -> p c d n", c=BC, d=D),
                        c_b)
                    # s += sel_bc^T w   (accumulate over blocks in PSUM)
                    for bc in range(BC):
                        for h in range(2):
                            nc.tensor.matmul(
                                s_ps[:, h * 512:(h + 1) * 512],
                                sel_sb[:, bc * 128:(bc + 1) * 128],
                                w_g[:, bc * ND + h * 512:bc * ND + (h + 1) * 512],
                                start=(blk == 0 and bc == 0),
                                stop=(blk == NB - 1 and bc == BC - 1),
                                skip_group_check=True)
                vb = reduce_squash_v(s_ps, 1.0, last=(it == 2))

    nc.compile()
    return nc


_CACHED = {}


def _get_program():
    if "nc" not in _CACHED:
        _CACHED["nc"] = _build_program()
    return _CACHED["nc"]


def kernel(x, W, bias):
    x = np.asarray(x, dtype=np.float32)
    W = np.asarray(W, dtype=np.float32)
    bias = np.asarray(bias, dtype=np.float32)

    w_all, xbd_all, xd_all, sels, bias_f = _host_prep(x, W, bias)
    nc = _get_program()

    in_maps = []
    for c in range(CORES):
        in_maps.append({
            "w_d": w_all[c],
            "xbd_d": xbd_all[c],
            "xd_d": xd_all[c],
            "sel_d": sels,
            "bias_d": bias_f,
        })
    res = run_bass_kernel_spmd(nc, in_maps, core_ids=list(range(CORES)))
    _CACHED["last_results"] = res
    # v_out is replicated; columns are (d,n) ordered -> [b, n, d]
    v = res.results[0]["v_out"].reshape(B, D, NCAP).transpose(0, 2, 1)
    return np.ascontiguousarray(v)


# revision 23
# speedup vs baseline: 1.2895x; 1.0405x over previous
"""ClassCapsule dynamic-routing kernel for 8 Trainium2 NeuronCores.

Problem (hardcoded shapes):
    x:    [64, 2048, 16]  fp32
    W:    [2048, 16, 1024] fp32
    bias: [64, 16]        fp32
    out:  [64, 64, 16]    fp32  (squeezed v after 3 routing iterations)

Strategy (in_caps-sharded, W resident in SBUF, u_hat recomputed per
iteration, per-iteration AllReduce of the small s tensor):
  - in_caps=2048 split across 8 cores (256 each); every core holds the
    full batch B=64.  W slice (bf16) lives in SBUF for the whole kernel,
    so u_hat is recomputed on the PE each routing iteration instead of
    being bounced through DRAM.  Total HBM traffic is ~15 MB/core.
  - u_hat tiles [128=(i8,b16), 1024=(d16,n64)] come from block-diagonal
    matmuls: lhsT = block-diag x (8 in_caps share K=128=(i8,e16)),
    rhs = W block.  Column order (d major, n minor) keeps the free-dim
    broadcast of c packed so DVE runs in 2x bf16 mode.
  - iteration 0 (uniform c): s0 = sum_i u/64 collapses into a dense
    x^T @ W matmul - no u_hat materialization at all.
  - routing: agreement = u*v reduced over d via a halving add tree
    (DVE, bf16), softmax over n (ACT exp + DVE), weighted sum over i
    via selector matmuls on the PE accumulating in PSUM.
  - s [64,1024] partials are AllReduced (collective_compute) across the
    8 cores each iteration; every core computes squash/v redundantly.
"""

import numpy as np
import ml_dtypes

import concourse.bass as bass
import concourse.tile as tile
from concourse import bacc, mybir
from concourse.bass_utils import run_bass_kernel_spmd

# ---------------------------------------------------------------- constants
B, IC, E = 64, 2048, 16          # batch, in_caps, in_dim
NCAP, D = 64, 16                 # n_caps, cap_dim
ND = NCAP * D                    # 1024
CORES = 8
ICL = IC // CORES                # 256 local in_caps
NB = ICL // 8                    # 32 blocks of 8 in_caps
BC = 4                           # batch chunks of 16
EPS = 1e-7

FP = mybir.dt.float32
BF = mybir.dt.bfloat16
BF_NP = ml_dtypes.bfloat16


def _host_prep(x, W, bias):
    """Per-core host-side tensors (bf16, (d,n) column order)."""
    # W columns reordered from (n,d) to (d,n): new_col = d*64 + n
    W_dn = W.reshape(IC, E, NCAP, D).transpose(0, 1, 3, 2).reshape(IC, E, ND)

    w_all, xbd_all, xd_all = [], [], []
    for c in range(CORES):
        sl = slice(c * ICL, (c + 1) * ICL)
        W_c = W_dn[sl]                                   # [256, 16, 1024]
        # -> [128=(i8,e16) partitions, 32 blocks, 1024]
        w_all.append(np.ascontiguousarray(
            W_c.reshape(NB, 8, E, ND).transpose(1, 2, 0, 3).reshape(128, NB, ND)
        ).astype(BF_NP))

        x_c = x[:, sl]                                   # [64, 256, 16]
        # block-diagonal lhsT: [128=(i8,e16), blk, bc, 128=(i8,b16)]
        x_r = x_c.reshape(BC, 16, NB, 8, E).transpose(3, 4, 2, 0, 1)
        arr = np.zeros((8, E, NB, BC, 8, 16), dtype=np.float32)
        for s in range(8):
            arr[s, :, :, :, s, :] = x_r[s]
        xbd_all.append(arr.reshape(128, NB, BC, 128).astype(BF_NP))

        # dense lhsT for iter-0 direct sum: [128=(i8,e16), blk, 128(m: b64 pad)]
        xd = np.zeros((128, NB, 128), dtype=np.float32)
        xd[:, :, :B] = x_c.reshape(B, NB, 8, E).transpose(2, 3, 1, 0).reshape(128, NB, B)
        xd_all.append(xd.astype(BF_NP))

    # selector weights, one per batch chunk: sel[bc][k=(i8,b16), m=128] with
    # m = bc*16 + (k%16) set to 1  (M=128 keeps free-weight-load enabled)
    sels = np.zeros((BC, 128, 128), dtype=np.float32)
    for bc in range(BC):
        k = np.arange(128)
        sels[bc, k, bc * 16 + (k % 16)] = 1.0
    sels = np.ascontiguousarray(sels.transpose(1, 0, 2)).astype(BF_NP)  # [128, BC, 128]

    # bias in (d,n) order, tiled over batch: [64, 1024]
    bias_dn = np.ascontiguousarray(bias.T).reshape(1, ND)       # [d,n] flat
    bias_f = np.tile(bias_dn, (B, 1)).astype(np.float32)
    return w_all, xbd_all, xd_all, sels, bias_f


def _build_program():
    nc = bacc.Bacc("TRN2", target_bir_lowering=False, num_devices=CORES)

    w_d = nc.dram_tensor("w_d", [128, NB, ND], BF, kind="ExternalInput")
    xbd_d = nc.dram_tensor("xbd_d", [128, NB, BC, 128], BF, kind="ExternalInput")
    xd_d = nc.dram_tensor("xd_d", [128, NB, 128], BF, kind="ExternalInput")
    sel_d = nc.dram_tensor("sel_d", [128, BC, 128], BF, kind="ExternalInput")
    bias_d = nc.dram_tensor("bias_d", [B, ND], FP, kind="ExternalInput")
    v_out = nc.dram_tensor("v_out", [B, ND], FP, kind="ExternalOutput")

    v_scr = nc.dram_tensor("v_scr", [B, ND], BF)     # bounce for vb build

    with tile.TileContext(nc) as tc:
        with (
            tc.tile_pool(name="consts", bufs=1) as cp,
            tc.tile_pool(name="ubf", bufs=3) as up,       # [128, 4096] grouped
            tc.tile_pool(name="tmp", bufs=3) as tp,       # [128, 4096] tmp+ws ring
            tc.tile_pool(name="vb", bufs=1) as vbp,
            tc.tile_pool(name="smalls", bufs=2) as sp,
            tc.tile_pool(name="sq", bufs=1) as qp,
            tc.tile_pool(name="ups", bufs=3, space="PSUM") as psp,
            tc.tile_pool(name="sps", bufs=1, space="PSUM") as psa,
            tc.tile_pool(name="bstate", bufs=1) as bsp,
            tc.tile_pool(name="dram", bufs=2, space="DRAM") as dp,
        ):
            # ---- resident tensors.  Load order matters: iter 0 needs xd +
            # w chunks; xbd is only needed at iter 1 so it loads last.
            xd_sb = up.tile([128, NB * 128], BF, tag="u_g")
            nc.sync.dma_start(out=xd_sb, in_=xd_d[:, :, :])
            sel_sb = cp.tile([128, BC * 128], BF)
            nc.sync.dma_start(out=sel_sb, in_=sel_d[:, :, :])
            bias_sb = cp.tile([B, ND], FP)
            nc.sync.dma_start(out=bias_sb, in_=bias_d[:, :])
            eps_t = cp.tile([B, 1], FP)
            nc.vector.memset(eps_t, EPS)
            w_sb = cp.tile([128, NB * ND], BF)
            WCH = 8  # blocks per load chunk; per-chunk deps let iter0 start early
            for ch in range(NB // WCH):
                nc.sync.dma_start(
                    out=w_sb[:, ch * WCH * ND:(ch + 1) * WCH * ND],
                    in_=w_d[:, ch * WCH:(ch + 1) * WCH, :])
            xbd_sb = cp.tile([128, NB * BC * 128], BF)
            nc.sync.dma_start(out=xbd_sb, in_=xbd_d[:, :, :, :])

            # warm up the collective path while inputs stream in: the first
            # AllReduce pays one-time channel setup, so do a tiny dummy one
            warm_in = dp.tile([B, 4], FP, tag="warm_in")
            warm_out = dp.tile([B, 4], FP, tag="warm_out")
            warm_sb = cp.tile([B, 4], FP)
            nc.vector.memset(warm_sb, 0.0)
            nc.sync.dma_start(out=warm_in[:], in_=warm_sb)
            nc.gpsimd.collective_compute(
                "AllReduce",
                mybir.AluOpType.add,
                replica_groups=[list(range(CORES))],
                ins=[warm_in[:].opt()],
                outs=[warm_out[:].opt()],
            )

            # routing logits state: [128=(i8,b16), bc*blk*n]
            b_all = bsp.tile([128, BC * NB * NCAP], FP)

            def w_blk(blk, h):
                return w_sb[:, blk * ND + h * 512:blk * ND + (h + 1) * 512]

            # ---------------- AllReduce s -> (scale,bias) -> squash -> v
            def reduce_squash_v(s_ps, scale, last):
                # AllReduce in bf16: halves the collective payload; the
                # ~0.4% rounding on s is well inside the error budget
                s_par = qp.tile([B, ND], BF, tag="q0")
                nc.vector.tensor_copy(out=s_par, in_=s_ps[0:B, :])
                s_in = dp.tile([B, ND], BF, tag="cc_in")
                nc.sync.dma_start(out=s_in[:], in_=s_par)
                s_red = dp.tile([B, ND], BF, tag="cc_out")
                nc.gpsimd.collective_compute(
                    "AllReduce",
                    mybir.AluOpType.add,
                    replica_groups=[list(range(CORES))],
                    ins=[s_in[:].opt()],
                    outs=[s_red[:].opt()],
                )
                s_glob = qp.tile([B, ND], BF, tag="q2b")
                nc.sync.dma_start(out=s_glob, in_=s_red[:])
                # s = s_glob*scale + bias
                s_sb = qp.tile([B, ND], FP, tag="q1")
                nc.vector.scalar_tensor_tensor(
                    out=s_sb, in0=s_glob, scalar=float(scale), in1=bias_sb,
                    op0=mybir.AluOpType.mult, op1=mybir.AluOpType.add)
                sqr = qp.tile([B, ND], FP, tag="q2")
                nc.scalar.square(out=sqr, in_=s_sb)
                nsq = sp.tile([B, NCAP], FP, tag="nsq")
                nc.vector.reduce_sum(
                    out=nsq, in_=sqr.rearrange("p (d n) -> p n d", d=D),
                    axis=mybir.AxisListType.X)
                norm = sp.tile([B, NCAP], FP, tag="norm")
                nc.scalar.activation(out=norm, in_=nsq,
                                     func=mybir.ActivationFunctionType.Sqrt,
                                     bias=eps_t[:, :], scale=1.0)
                den = sp.tile([B, NCAP], FP, tag="den")
                nc.vector.scalar_tensor_tensor(
                    out=den, in0=nsq, scalar=float(EPS + 1.0), in1=norm,
                    op0=mybir.AluOpType.add, op1=mybir.AluOpType.mult)
                rden = sp.tile([B, NCAP], FP, tag="rden")
                nc.vector.reciprocal(out=rden, in_=den)
                fac = sp.tile([B, NCAP], FP, tag="fac")
                nc.vector.scalar_tensor_tensor(
                    out=fac, in0=nsq, scalar=float(EPS), in1=rden,
                    op0=mybir.AluOpType.add, op1=mybir.AluOpType.mult)
                v_sb = qp.tile([B, ND], FP, tag="q2")
                fac_b = bass.AP(tensor=fac.tensor, offset=fac.offset,
                                ap=[list(fac.ap[0]), [0, D], list(fac.ap[1])])
                nc.vector.tensor_mul(
                    v_sb.rearrange("p (d n) -> p d n", d=D),
                    s_sb.rearrange("p (d n) -> p d n", d=D),
                    fac_b)
                if last:
                    nc.sync.dma_start(out=v_out[:, :], in_=v_sb)
                    return None
                v_bf = qp.tile([B, ND], BF, tag="q0")
                nc.vector.tensor_copy(out=v_bf, in_=v_sb)
                nc.sync.dma_start(out=v_scr[:, :], in_=v_bf)
                # vb_all[128=(i8,b16), (bc,d,n)]: v rows bc*16+b_lo, bcast i8
                vb = vbp.tile([128, BC * ND], BF, tag="vb")
                for bc in range(BC):
                    src = bass.AP(tensor=v_scr, offset=bc * 16 * ND,
                                  ap=[[0, 8], [ND, 16], [1, ND]])
                    nc.sync.dma_start(
                        out=vb[:, bc * ND:(bc + 1) * ND], in_=src)
                return vb

            # ================= iter 0: s0 = (1/64) sum_i u  ================
            s_ps = psa.tile([128, ND], FP, tag="s_acc")
            for blk in range(NB):
                for h in range(2):
                    nc.tensor.matmul(
                        s_ps[:, h * 512:(h + 1) * 512],
                        xd_sb[:, blk * 128:(blk + 1) * 128],
                        w_blk(blk, h),
                        start=(blk == 0), stop=(blk == NB - 1))
            vb = reduce_squash_v(s_ps, 1.0 / NCAP, last=False)

            # ================= routing iterations 1 and 2 =================
            for it in (1, 2):
                s_ps = psa.tile([128, ND], FP, tag="s_acc")
                for blk in range(NB):
                    u_g = up.tile([128, BC * ND], BF, tag="u_g")
                    for bc in range(BC):
                        u_ps = psp.tile([128, ND], FP, tag="u_ps")
                        lhs = xbd_sb[:, (blk * BC + bc) * 128:
                                     (blk * BC + bc + 1) * 128]
                        for h in range(2):
                            nc.tensor.matmul(
                                u_ps[:, h * 512:(h + 1) * 512],
                                lhs, w_blk(blk, h),
                                start=True, stop=True)
                        nc.scalar.copy(
                            out=u_g[:, bc * ND:(bc + 1) * ND], in_=u_ps)
                    # tmp = u * v  (bf16, packed -> 2x DVE)
                    tmp = tp.tile([128, BC * ND], BF, tag="tmp")
                    nc.vector.tensor_mul(tmp, u_g, vb)
                    # reduce over d: halving add tree on [p, bc, (d n)]
                    t3 = tmp.rearrange("p (c f) -> p c f", c=BC)
                    for half in (512, 256, 128, 64):
                        src_hi = bass.AP(
                            tensor=tmp.tensor, offset=tmp.offset + half,
                            ap=[list(tmp.ap[0]), [ND, BC], [1, half]])
                        if half > 64:
                            nc.vector.tensor_add(
                                t3[:, :, 0:half], t3[:, :, 0:half], src_hi)
                        else:
                            # final add -> b state (fp32)
                            b_dst = bass.AP(
                                tensor=b_all.tensor,
                                offset=b_all.offset + blk * NCAP,
                                ap=[list(b_all.ap[0]), [NB * NCAP, BC],
                                    [1, NCAP]])
                            if it == 1:
                                nc.vector.tensor_add(
                                    b_dst, t3[:, :, 0:64], src_hi)
                            else:
                                agr = sp.tile([128, BC * NCAP], FP, tag="agr")
                                a3 = agr.rearrange("p (c n) -> p c n", c=BC)
                                nc.vector.tensor_add(
                                    a3, t3[:, :, 0:64], src_hi)
                                nc.vector.tensor_add(b_dst, b_dst, a3)
                    # softmax over n (free axis): exp, Z, recip, scale
                    b_src = bass.AP(
                        tensor=b_all.tensor,
                        offset=b_all.offset + blk * NCAP,
                        ap=[list(b_all.ap[0]), [NB * NCAP, BC], [1, NCAP]])
                    c_un = sp.tile([128, BC * NCAP], BF, tag="c_un")
                    nc.scalar.activation(
                        out=c_un.rearrange("p (c n) -> p c n", c=BC),
                        in_=b_src, func=mybir.ActivationFunctionType.Exp)
                    zsum = sp.tile([128, BC], FP, tag="zsum")
                    nc.vector.reduce_sum(
                        out=zsum, in_=c_un.rearrange("p (c n) -> p c n", c=BC),
                        axis=mybir.AxisListType.X)
                    rec = sp.tile([128, BC], BF, tag="rec")
                    with nc.allow_low_precision(reason="1/Z in bf16 is fine for softmax scale"):
                        nc.vector.reciprocal(out=rec, in_=zsum)
                    c_bf = sp.tile([128, BC * NCAP], BF, tag="c_bf")
                    rec_b = bass.AP(tensor=rec.tensor, offset=rec.offset,
                                    ap=[list(rec.ap[0]), [1, BC], [0, NCAP]])
                    nc.gpsimd.tensor_mul(
                        c_bf.rearrange("p (c n) -> p c n", c=BC),
                        c_un.rearrange("p (c n) -> p c n", c=BC),
                        rec_b)
                    # w = u * c (c bcast over d; last dim packed -> 2x DVE)
                    w_g = tp.tile([128, BC * ND], BF, tag="tmp")
                    c_b = bass.AP(tensor=c_bf.tensor, offset=c_bf.offset,
                                  ap=[list(c_bf.ap[0]), [NCAP, BC], [0, D],
                                      [1, NCAP]])
                    nc.vector.tensor_mul(
                        w_g.rearrange("p (c d n) -> p c d n", c=BC, d=D),
                        u_g.rearrange("p (c d n) -> p c d n", c=BC, d=D),
                        c_b)
                    # s += sel_bc^T w   (accumulate over blocks in PSUM)
                    for bc in range(BC):
                        for h in range(2):
                            nc.tensor.matmul(
                                s_ps[:, h * 512:(h + 1) * 512],
                                sel_sb[:, bc * 128:(bc + 1) * 128],
                                w_g[:, bc * ND + h * 512:bc * ND + (h + 1) * 512],
                                start=(blk == 0 and bc == 0),
                                stop=(blk == NB - 1 and bc == BC - 1),
                                skip_group_check=True)
                vb = reduce_squash_v(s_ps, 1.0, last=(it == 2))

    nc.compile()
    return nc


_CACHED = {}


def _get_program():
    if "nc" not in _CACHED:
        _CACHED["nc"] = _build_program()
    return _CACHED["nc"]


def kernel(x, W, bias):
    x = np.asarray(x, dtype=np.float32)
    W = np.asarray(W, dtype=np.float32)
    bias = np.asarray(bias, dtype=np.float32)

    w_all, xbd_all, xd_all, sels, bias_f = _host_prep(x, W, bias)
    nc = _get_program()

    in_maps = []
    for c in range(CORES):
        in_maps.append({
            "w_d": w_all[c],
            "xbd_d": xbd_all[c],
            "xd_d": xd_all[c],
            "sel_d": sels,
            "bias_d": bias_f,
        })
    res = run_bass_kernel_spmd(nc, in_maps, core_ids=list(range(CORES)))
    _CACHED["last_results"] = res
    # v_out is replicated; columns are (d,n) ordered -> [b, n, d]
    v = res.results[0]["v_out"].reshape(B, D, NCAP).transpose(0, 2, 1)
    return np.ascontiguousarray(v)
